# revision 13
# baseline (speedup 1.0000x reference)
"""Trainium2 Bass kernel for the DeepSets-style segment_reduce network.

Network (per sample, B=2048, M=128 elements):
  phi: 3 -> 120 -> 100 -> 80 MLP (all ReLU), applied per element
  pooled = sum over the 128 elements                      [B, 80]
  rho:  80 -> 60 -> 60 -> 40 (ReLU, ReLU, linear)
  q:    concat(rho_out, static) 43 -> 200 -> 100 -> 3, softmax

Mapping: data-parallel over 8 NeuronCores (256 samples each). Activations are
kept feature-major [features, elements] in SBUF so every layer is a single
stationary-weight matmul; biases are folded in as an extra contraction row
driven by a constant-ones row. All matmul operands are fp16 (fp32 PSUM
accumulation). L1 (K=4) runs 4-way row-group packed on the PE array. The
element-pool runs as a fused relu+pairwise-add (scalar_tensor_tensor) off
PSUM, then fp16 halving trees on GpSimd. The rho/q/softmax tail runs per
128-sample half so the first half's tail hides under the second half's phi.
"""

import sys
import numpy as np

sys.path.insert(0, '/opt/trn_rl_repo')

B, M, D = 2048, 128, 3
N_CORES = 8
BC = B // N_CORES            # samples per core (256)
EC = BC * M                  # elements per core (32768)
HALF = EC // 2               # elements per half-sweep (16384)
SAMP_HALF = BC // 2          # samples per half (128)
PT = 1024                    # elements per PSUM tile
NPT = HALF // PT             # psum tiles per half (16)
CPM = PT // 512              # 512-chunks per psum tile (2)
SPL = PT // 2                # ACT/DVE column split inside a [*, PT] copy
GS = M // 2                  # pair-summed group width (64)

F1, F2, F3 = 120, 100, 80    # phi widths
R1, R2, R3 = 60, 60, 40      # rho widths
Q1, Q2, Q3 = 200, 100, 3     # q widths
XQ = R3 + 3 + 1              # q input rows: rho_out + static + ones (44)

# packed weight blob column layout: name -> (rows, cols, col_offset)
_BLOB = {}
_off = 0
for _name, _r, _c in [("w1a", 4, F1), ("w2a", F1 + 1, F2), ("w3a", F2 + 1, F3),
                      ("r1a", F3 + 1, R1), ("r2a", R1 + 1, R2), ("r3a", R2 + 1, R3),
                      ("q1aw", XQ, 128), ("q1bw", XQ, Q1 - 128),
                      ("q2aw", 128, Q2), ("q2bw", Q1 - 128 + 1, Q2),
                      ("q3aw", Q2 + 1, Q3), ("statt", 3, BC)]:
    _BLOB[_name] = (_r, _c, _off)
    _off += _c
BLOBW = _off

_compiled = {}


def _build():
    import concourse.bacc as bacc
    import concourse.mybir as mybir
    from concourse import tile

    f32 = mybir.dt.float32
    fp16 = mybir.dt.float16
    Relu = mybir.ActivationFunctionType.Relu
    Exp = mybir.ActivationFunctionType.Exp
    Alu = mybir.AluOpType

    nc = bacc.Bacc("TRN2", target_bir_lowering=False, debug=False,
                   enable_asserts=False, num_devices=N_CORES)

    xin = nc.dram_tensor("xin", [2, 4, 4, HALF // 4], fp16, kind="ExternalInput").ap()
    blob = nc.dram_tensor("blob", [128, BLOBW], fp16, kind="ExternalInput").ap()
    onesr = nc.dram_tensor("onesr", [1, HALF], fp16, kind="ExternalInput").ap()
    eye3 = nc.dram_tensor("eye3", [3, 3], f32, kind="ExternalInput").ap()
    out = nc.dram_tensor("out", [BC, 3], f32, kind="ExternalOutput").ap()

    with tile.TileContext(nc) as tc:
        with tc.tile_pool(name="cst", bufs=1) as cst, \
             tc.tile_pool(name="xp", bufs=2) as xp, \
             tc.tile_pool(name="scr", bufs=1) as scr, \
             tc.tile_pool(name="ps", bufs=3, space="PSUM") as ps, \
             tc.tile_pool(name="pst", bufs=2, space="PSUM") as pst:

            # x half 0 first so the PE can start ASAP (4 row-group DMAs)
            x_sb0 = xp.tile([128, HALF // 4], fp16, name="x_sb0", tag="x", bufs=2)
            for j in range(4):
                eng = nc.sync if j < 2 else nc.scalar
                eng.dma_start(out=x_sb0[32 * j:32 * j + 4, :], in_=xin[0, j])

            blob_sb = cst.tile([128, BLOBW], fp16)
            nc.sync.dma_start(out=blob_sb[:, :], in_=blob)

            def wslice(name):
                r, c, o = _BLOB[name]
                return blob_sb[0:r, o:o + c]

            w2s, w3s = wslice("w2a"), wslice("w3a")
            r1s, r2s, r3s = wslice("r1a"), wslice("r2a"), wslice("r3a")
            q1as, q1bs = wslice("q1aw"), wslice("q1bw")
            q2as, q2bs = wslice("q2aw"), wslice("q2bw")
            q3as, statt = wslice("q3aw"), wslice("statt")

            # PE warm-up source (HAM ramp while input DMAs land)
            wtiny = cst.tile([128, 512], fp16)
            nc.vector.memset(wtiny[:, :], 0.0)
            tpre = cst.tile([1, 2], fp16)
            nc.scalar.activation(tpre[:, :], wtiny[0:1, 0:2],
                                 mybir.ActivationFunctionType.Relu)

            # persistent activation planes (ones rows written once)
            h1 = cst.tile([F1 + 1, HALF], fp16)
            h2 = cst.tile([F2 + 1, HALF], fp16)
            s_half = cst.tile([F3, HALF // 2], fp16)
            nc.gpsimd.dma_start(out=h1[F1:F1 + 1, :], in_=onesr)
            nc.gpsimd.dma_start(out=h2[F2:F2 + 1, :], in_=onesr)

            pooled = cst.tile([F3 + 1, BC], fp16)      # ones row at 80
            nc.gpsimd.dma_start(out=pooled[F3:F3 + 1, :], in_=onesr[:, 0:BC])
            eye3s = cst.tile([3, 3], f32)
            nc.gpsimd.dma_start(out=eye3s[:, :], in_=eye3)
            eye1 = cst.tile([1, 1], f32)
            nc.vector.memset(eye1[:, :], 1.0)
            ones3 = cst.tile([3, 1], f32)
            nc.vector.memset(ones3[:, :], 1.0)

            # per-half tail tiles (ones rows preloaded)
            xqs, hr1s, hr2s, hq1as, hq1bs, hq2s, e_sbs = [], [], [], [], [], [], []
            for h in range(2):
                xqh = cst.tile([XQ, SAMP_HALF], fp16, name=f"xq{h}")
                nc.gpsimd.dma_start(out=xqh[R3:R3 + 3, :],
                                    in_=statt[:, h * SAMP_HALF:(h + 1) * SAMP_HALF])
                nc.gpsimd.dma_start(out=xqh[XQ - 1:XQ, :], in_=onesr[:, 0:SAMP_HALF])
                xqs.append(xqh)
                hr1h = cst.tile([R1 + 1, SAMP_HALF], fp16, name=f"hr1{h}")
                nc.gpsimd.dma_start(out=hr1h[R1:R1 + 1, :], in_=onesr[:, 0:SAMP_HALF])
                hr1s.append(hr1h)
                hr2h = cst.tile([R2 + 1, SAMP_HALF], fp16, name=f"hr2{h}")
                nc.gpsimd.dma_start(out=hr2h[R2:R2 + 1, :], in_=onesr[:, 0:SAMP_HALF])
                hr2s.append(hr2h)
                hq1as.append(cst.tile([128, SAMP_HALF], fp16, name=f"hq1a{h}"))
                hq1bh = cst.tile([Q1 - 128 + 1, SAMP_HALF], fp16, name=f"hq1b{h}")
                nc.gpsimd.dma_start(out=hq1bh[Q1 - 128:Q1 - 128 + 1, :],
                                    in_=onesr[:, 0:SAMP_HALF])
                hq1bs.append(hq1bh)
                hq2h = cst.tile([Q2 + 1, SAMP_HALF], fp16, name=f"hq2{h}")
                nc.gpsimd.dma_start(out=hq2h[Q2:Q2 + 1, :], in_=onesr[:, 0:SAMP_HALF])
                hq2s.append(hq2h)
                e_sbs.append(cst.tile([3, SAMP_HALF], f32, name=f"e_sb{h}"))

            # warm-up matmuls (output consumed by a cheap DVE op)
            pw = ps.tile([128, 512], f32, name="pw", tag="hp")
            for i in range(8):
                nc.tensor.matmul(pw[:, :], wtiny[:, 0:128], wtiny[:, :],
                                 start=True, stop=True)
            wsink = cst.tile([32, 2], fp16)
            nc.vector.tensor_scalar_max(wsink[:, :], pw[0:32, 0:2], 0.0)

            def tail_half(h):
                sl = slice(h * SAMP_HALF, (h + 1) * SAMP_HALF)
                xqh, hr1h, hr2h = xqs[h], hr1s[h], hr2s[h]
                hq1ah, hq1bh, hq2h, e_sb = hq1as[h], hq1bs[h], hq2s[h], e_sbs[h]

                pr1 = pst.tile([R1, SAMP_HALF], f32, name=f"pr1_{h}", tag="tail")
                nc.tensor.matmul(pr1[:, :], r1s, pooled[:, sl], start=True, stop=True)
                nc.scalar.activation(hr1h[0:R1, :], pr1[:, :], Relu)

                pr2 = pst.tile([R2, SAMP_HALF], f32, name=f"pr2_{h}", tag="tail")
                nc.tensor.matmul(pr2[:, :], r2s, hr1h[:, :], start=True, stop=True)
                nc.scalar.activation(hr2h[0:R2, :], pr2[:, :], Relu)

                pr3 = pst.tile([R3, SAMP_HALF], f32, name=f"pr3_{h}", tag="tail")
                nc.tensor.matmul(pr3[:, :], r3s, hr2h[:, :], start=True, stop=True)
                nc.scalar.copy(xqh[0:R3, :], pr3[:, :])

                pq1a = pst.tile([128, SAMP_HALF], f32, name=f"pq1a_{h}", tag="tail")
                pq1b = pst.tile([Q1 - 128, SAMP_HALF], f32, name=f"pq1b_{h}", tag="tail")
                nc.tensor.matmul(pq1a[:, :], q1as, xqh[:, :], start=True, stop=True)
                nc.tensor.matmul(pq1b[:, :], q1bs, xqh[:, :], start=True, stop=True)
                nc.scalar.activation(hq1ah[:, :], pq1a[:, :], Relu)
                nc.vector.tensor_scalar_max(hq1bh[0:Q1 - 128, :], pq1b[:, :], 0.0)

                pq2 = pst.tile([Q2, SAMP_HALF], f32, name=f"pq2_{h}", tag="tail")
                nc.tensor.matmul(pq2[:, :], q2as, hq1ah[:, :], start=True, stop=False)
                nc.tensor.matmul(pq2[:, :], q2bs, hq1bh[:, :], start=False, stop=True)
                nc.scalar.activation(hq2h[0:Q2, :], pq2[:, :], Relu)

                pq3 = pst.tile([Q3, SAMP_HALF], f32, name=f"pq3_{h}", tag="tail")
                nc.tensor.matmul(pq3[:, :], q3as, hq2h[:, :], start=True, stop=True)

                # softmax (fp32)
                nc.scalar.activation(e_sb[:, :], pq3[:, :], Exp)
                ssum = pst.tile([1, SAMP_HALF], f32, name=f"ssum{h}", tag="tail")
                nc.tensor.matmul(ssum[:, :], ones3[:, :], e_sb[:, :], start=True, stop=True)
                rec = cst.tile([1, SAMP_HALF], f32, name=f"rec{h}")
                nc.vector.reciprocal(rec[:, :], ssum[:, :])

                eT = pst.tile([128, 3], f32, name=f"eT{h}", tag="tail")
                nc.tensor.transpose(eT[:, :], e_sb[:, :], eye3s[:, :])
                rT = pst.tile([128, 1], f32, name=f"rT{h}", tag="tail")
                nc.tensor.transpose(rT[:, :], rec[:, :], eye1[:, :])
                rTs = cst.tile([128, 1], f32, name=f"rTs{h}")
                nc.vector.tensor_copy(rTs[:, :], rT[:, :])
                o_sb = cst.tile([128, 3], f32, name=f"o_sb{h}")
                nc.vector.tensor_scalar_mul(o_sb[:, :], eT[:, :], rTs[:, :])
                nc.sync.dma_start(out=out[h * SAMP_HALF:(h + 1) * SAMP_HALF, :],
                                  in_=o_sb[:, :])

            def phi_l1(h, x_sb):
                # L1 sweep: CPM chunks per tile, one per PE row-group (concurrent)
                for t in range(NPT):
                    p1 = ps.tile([128, PT], f32, name="p1", tag="hp")
                    for cc in range(CPM):
                        ch = t * CPM + cc             # global 512-chunk index
                        j = ch % 4                     # PE row-group
                        g = ch // 4                    # x column group
                        nc.tensor.matmul(p1[0:F1, cc * 512:(cc + 1) * 512],
                                         blob_sb[32 * j:32 * j + 4, 0:F1],
                                         x_sb[32 * j:32 * j + 4, g * 512:(g + 1) * 512],
                                         start=True, stop=True, tile_position=(32 * j, 0))
                    c = t * PT
                    if (t % 8) in (0, 2, 4, 6, 7):
                        nc.scalar.activation(h1[0:F1, c:c + PT], p1[0:F1, :], Relu)
                    else:
                        nc.vector.tensor_scalar_max(h1[0:F1, c:c + PT], p1[0:F1, :], 0.0)
            def phi_l2(h):
                for t in range(NPT):
                    p2 = ps.tile([128, PT], f32, name="p2", tag="hp")
                    for cc in range(CPM):
                        c0 = t * PT + cc * 512
                        nc.tensor.matmul(p2[0:F2, cc * 512:(cc + 1) * 512],
                                         w2s, h1[:, c0:c0 + 512],
                                         start=True, stop=True)
                    c = t * PT
                    if (t % 8) in (0, 2, 4, 6, 7):
                        nc.scalar.activation(h2[0:F2, c:c + PT], p2[0:F2, :], Relu)
                    else:
                        nc.vector.tensor_scalar_max(h2[0:F2, c:c + PT], p2[0:F2, :], 0.0)
            def phi_l3(h):
                for t in range(NPT):
                    p3 = ps.tile([128, PT], f32, name="p3", tag="hp")
                    for cc in range(CPM):
                        c0 = t * PT + cc * 512
                        nc.tensor.matmul(p3[0:F3, cc * 512:(cc + 1) * 512],
                                         w3s, h2[:, c0:c0 + 512],
                                         start=True, stop=True)
                    p3g = p3[0:F3, :].rearrange("p (g m) -> p g m", m=M)
                    h3ra = scr.tile([F3, PT // 2], fp16, name="h3ra", tag="h3ra", bufs=3)
                    h3g = h3ra[:, :].rearrange("p (g m) -> p g m", m=GS)
                    # first half of each 128-group: plain relu copy (ACT)
                    nc.scalar.activation(h3g, p3g[:, :, 0:GS], Relu)
                    # second half: relu(psum) + h3ra -> pairwise-summed (DVE)
                    sv = s_half[:, t * (PT // 2):(t + 1) * (PT // 2)]
                    nc.vector.scalar_tensor_tensor(
                        sv.rearrange("p (g m) -> p g m", m=GS),
                        p3g[:, :, GS:M], 0.0, h3g,
                        op0=Alu.max, op1=Alu.add)
            def trees(h):
                USAMP = 32
                for u in range(SAMP_HALF // USAMP):
                    w = GS
                    srt = s_half[:, u * USAMP * GS:(u + 1) * USAMP * GS]
                    lv = 0
                    while w > 1:
                        w //= 2
                        if w > 1:
                            nxt = scr.tile([F3, USAMP * w], fp16, name=f"tl{lv}",
                                           tag=f"tl{lv}", bufs=2)[:, :]
                        else:
                            nxt = pooled[0:F3, h * SAMP_HALF + u * USAMP:
                                         h * SAMP_HALF + (u + 1) * USAMP]
                        a3 = srt.rearrange("p (g m) -> p g m", m=2 * w)
                        eng = nc.gpsimd if lv == 0 else nc.vector
                        eng.tensor_tensor(
                            out=nxt.rearrange("p (g m) -> p g m", m=w) if w > 1 else nxt,
                            in0=a3[:, :, 0:w], in1=a3[:, :, w:2 * w], op=Alu.add)
                        srt = nxt
                        lv += 1

            phi_l1(0, x_sb0)
            phi_l2(0)
            phi_l3(0)
            trees(0)
            x_sb1 = xp.tile([128, HALF // 4], fp16, name="x_sb1", tag="x", bufs=2)
            for j in range(4):
                eng = nc.sync if j < 2 else nc.scalar
                eng.dma_start(out=x_sb1[32 * j:32 * j + 4, :], in_=xin[1, j])
            phi_l1(1, x_sb1)
            tail_half(0)
            phi_l2(1)
            phi_l3(1)
            trees(1)
            tail_half(1)

    nc.compile()
    return nc


def _prep_inputs(dyn, static, phi_w1, phi_b1, phi_w2, phi_b2, phi_w3, phi_b3,
                 rho_w1, rho_b1, rho_w2, rho_b2, rho_w3, rho_b3,
                 q_w1, q_b1, q_w2, q_b2, q_w3, q_b3):
    """Build the per-core input maps (host-side layout transforms, all fp16)."""
    fp16 = np.float16

    def aug_t(w, b):
        # [out, in] weight + bias -> transposed augmented [in+1, out]
        return np.concatenate([w, b[:, None]], axis=1).T.astype(fp16)

    q1 = aug_t(q_w1, q_b1)               # [44, 200]
    q2 = aug_t(q_w2, q_b2)               # [201, 100]
    parts = dict(
        w1a=aug_t(phi_w1, phi_b1), w2a=aug_t(phi_w2, phi_b2),
        w3a=aug_t(phi_w3, phi_b3), r1a=aug_t(rho_w1, rho_b1),
        r2a=aug_t(rho_w2, rho_b2), r3a=aug_t(rho_w3, rho_b3),
        q1aw=q1[:, 0:128], q1bw=q1[:, 128:],
        q2aw=q2[0:128, :], q2bw=q2[128:, :], q3aw=aug_t(q_w3, q_b3))

    eye3 = np.eye(3, dtype=np.float32)
    onesr = np.ones((1, HALF), dtype=fp16)

    base_blob = np.zeros((128, BLOBW), dtype=fp16)
    for name, (r, cc, o) in _BLOB.items():
        if name != "statt":
            base_blob[0:r, o:o + cc] = parts[name]
    for j in range(1, 4):   # replicate L1 weights into each PE row-group
        base_blob[32 * j:32 * j + 4, 0:F1] = parts["w1a"]

    in_maps = []
    for c in range(N_CORES):
        blob = base_blob.copy()
        r, cc, o = _BLOB["statt"]
        blob[0:r, o:o + cc] = static[c * BC:(c + 1) * BC].T.astype(fp16)
        xc = dyn[c * BC:(c + 1) * BC].reshape(EC, D).astype(fp16)
        # [2 halves, 4 row-groups, 4 rows(x0,x1,x2,1), HALF//4]
        xin = np.empty((2, 4, 4, HALF // 4), dtype=fp16)
        for hh in range(2):
            xh = xc[hh * HALF:(hh + 1) * HALF].reshape(HALF // 512, 512, D)
            for j in range(4):
                chunks = xh[j::4]                      # [8, 512, 3]
                blkT = chunks.transpose(2, 0, 1).reshape(D, -1)
                xin[hh, j, 0:3] = blkT
                xin[hh, j, 3] = 1.0
        in_maps.append(dict(xin=xin, blob=blob, onesr=onesr, eye3=eye3))
    return in_maps


def kernel(**inputs):
    from concourse.bass_utils import run_bass_kernel_spmd

    if "nc" not in _compiled:
        _compiled["nc"] = _build()
    nc = _compiled["nc"]

    in_maps = _prep_inputs(**inputs)
    res = run_bass_kernel_spmd(nc, in_maps, core_ids=list(range(N_CORES)))
    out = np.concatenate([res.results[c]["out"] for c in range(N_CORES)], axis=0)
    return out.astype(np.float32)


# revision 14
# speedup vs baseline: 1.0233x; 1.0233x over previous
"""Trainium2 Bass kernel for the DeepSets-style segment_reduce network.

Network (per sample, B=2048, M=128 elements):
  phi: 3 -> 120 -> 100 -> 80 MLP (all ReLU), applied per element
  pooled = sum over the 128 elements                      [B, 80]
  rho:  80 -> 60 -> 60 -> 40 (ReLU, ReLU, linear)
  q:    concat(rho_out, static) 43 -> 200 -> 100 -> 3, softmax

Mapping: data-parallel over 8 NeuronCores (256 samples each). Activations are
kept feature-major [features, elements] in SBUF so every layer is a single
stationary-weight matmul; biases are folded in as an extra contraction row
driven by a constant-ones row. All matmul operands are fp16 (fp32 PSUM
accumulation). L1 (K=4) runs 4-way row-group packed on the PE array. The
element-pool runs as a fused relu+pairwise-add (scalar_tensor_tensor) off
PSUM, then fp16 halving trees on GpSimd. The rho/q/softmax tail runs per
128-sample half so the first half's tail hides under the second half's phi.
"""

import sys
import numpy as np

sys.path.insert(0, '/opt/trn_rl_repo')

B, M, D = 2048, 128, 3
N_CORES = 8
BC = B // N_CORES            # samples per core (256)
EC = BC * M                  # elements per core (32768)
HALF = EC // 2               # elements per half-sweep (16384)
SAMP_HALF = BC // 2          # samples per half (128)
PT = 1024                    # elements per PSUM tile
NPT = HALF // PT             # psum tiles per half (16)
CPM = PT // 512              # 512-chunks per psum tile (2)
SPL = PT // 2                # ACT/DVE column split inside a [*, PT] copy
GS = M // 2                  # pair-summed group width (64)

F1, F2, F3 = 120, 100, 80    # phi widths
R1, R2, R3 = 60, 60, 40      # rho widths
Q1, Q2, Q3 = 200, 100, 3     # q widths
XQ = R3 + 3 + 1              # q input rows: rho_out + static + ones (44)

# packed weight blob column layout: name -> (rows, cols, col_offset)
_BLOB = {}
_off = 0
for _name, _r, _c in [("w1a", 4, F1), ("w2a", F1 + 1, F2), ("w3a", F2 + 1, F3),
                      ("r1a", F3 + 1, R1), ("r2a", R1 + 1, R2), ("r3a", R2 + 1, R3),
                      ("q1aw", XQ, 128), ("q1bw", XQ, Q1 - 128),
                      ("q2aw", 128, Q2), ("q2bw", Q1 - 128 + 1, Q2),
                      ("q3aw", Q2 + 1, Q3), ("statt", 3, BC)]:
    _BLOB[_name] = (_r, _c, _off)
    _off += _c
BLOBW = _off

_compiled = {}


def _build():
    import concourse.bacc as bacc
    import concourse.mybir as mybir
    from concourse import tile

    f32 = mybir.dt.float32
    fp16 = mybir.dt.float16
    Relu = mybir.ActivationFunctionType.Relu
    Exp = mybir.ActivationFunctionType.Exp
    Alu = mybir.AluOpType

    nc = bacc.Bacc("TRN2", target_bir_lowering=False, debug=False,
                   enable_asserts=False, num_devices=N_CORES)

    xin = nc.dram_tensor("xin", [2, 4, 4, HALF // 4], fp16, kind="ExternalInput").ap()
    blob = nc.dram_tensor("blob", [128, BLOBW], fp16, kind="ExternalInput").ap()
    onesr = nc.dram_tensor("onesr", [1, HALF], fp16, kind="ExternalInput").ap()
    eye3 = nc.dram_tensor("eye3", [3, 3], f32, kind="ExternalInput").ap()
    out = nc.dram_tensor("out", [BC, 3], f32, kind="ExternalOutput").ap()

    with tile.TileContext(nc) as tc:
        with tc.tile_pool(name="cst", bufs=1) as cst, \
             tc.tile_pool(name="xp", bufs=2) as xp, \
             tc.tile_pool(name="scr", bufs=1) as scr, \
             tc.tile_pool(name="ps", bufs=3, space="PSUM") as ps, \
             tc.tile_pool(name="pst", bufs=2, space="PSUM") as pst:

            # x half 0 first so the PE can start ASAP (4 row-group DMAs)
            x_sb0 = xp.tile([128, HALF // 4], fp16, name="x_sb0", tag="x", bufs=2)
            for j in range(4):
                eng = nc.sync if j < 2 else nc.scalar
                eng.dma_start(out=x_sb0[32 * j:32 * j + 4, :], in_=xin[0, j])

            blob_sb = cst.tile([128, BLOBW], fp16)
            nc.sync.dma_start(out=blob_sb[:, :], in_=blob)

            def wslice(name):
                r, c, o = _BLOB[name]
                return blob_sb[0:r, o:o + c]

            w2s, w3s = wslice("w2a"), wslice("w3a")
            r1s, r2s, r3s = wslice("r1a"), wslice("r2a"), wslice("r3a")
            q1as, q1bs = wslice("q1aw"), wslice("q1bw")
            q2as, q2bs = wslice("q2aw"), wslice("q2bw")
            q3as, statt = wslice("q3aw"), wslice("statt")

            # PE warm-up source (HAM ramp while input DMAs land)
            wtiny = cst.tile([128, 512], fp16)
            nc.vector.memset(wtiny[:, :], 0.0)
            tpre = cst.tile([1, 2], fp16)
            nc.scalar.activation(tpre[:, :], wtiny[0:1, 0:2],
                                 mybir.ActivationFunctionType.Relu)

            # persistent activation planes (ones rows written once)
            h1 = cst.tile([F1 + 1, HALF], fp16)
            h2 = cst.tile([F2 + 1, HALF], fp16)
            s_half = cst.tile([F3, HALF // 2], fp16)
            nc.gpsimd.dma_start(out=h1[F1:F1 + 1, :], in_=onesr)
            nc.gpsimd.dma_start(out=h2[F2:F2 + 1, :], in_=onesr)

            pooled = cst.tile([F3 + 1, BC], fp16)      # ones row at 80
            nc.gpsimd.dma_start(out=pooled[F3:F3 + 1, :], in_=onesr[:, 0:BC])
            eye3s = cst.tile([3, 3], f32)
            nc.gpsimd.dma_start(out=eye3s[:, :], in_=eye3)
            eye1 = cst.tile([1, 1], f32)
            nc.vector.memset(eye1[:, :], 1.0)
            ones3 = cst.tile([3, 1], f32)
            nc.vector.memset(ones3[:, :], 1.0)

            # per-half tail tiles (ones rows preloaded)
            xqs, hr1s, hr2s, hq1as, hq1bs, hq2s, e_sbs = [], [], [], [], [], [], []
            for h in range(2):
                xqh = cst.tile([XQ, SAMP_HALF], fp16, name=f"xq{h}")
                nc.gpsimd.dma_start(out=xqh[R3:R3 + 3, :],
                                    in_=statt[:, h * SAMP_HALF:(h + 1) * SAMP_HALF])
                nc.gpsimd.dma_start(out=xqh[XQ - 1:XQ, :], in_=onesr[:, 0:SAMP_HALF])
                xqs.append(xqh)
                hr1h = cst.tile([R1 + 1, SAMP_HALF], fp16, name=f"hr1{h}")
                nc.gpsimd.dma_start(out=hr1h[R1:R1 + 1, :], in_=onesr[:, 0:SAMP_HALF])
                hr1s.append(hr1h)
                hr2h = cst.tile([R2 + 1, SAMP_HALF], fp16, name=f"hr2{h}")
                nc.gpsimd.dma_start(out=hr2h[R2:R2 + 1, :], in_=onesr[:, 0:SAMP_HALF])
                hr2s.append(hr2h)
                hq1as.append(cst.tile([128, SAMP_HALF], fp16, name=f"hq1a{h}"))
                hq1bh = cst.tile([Q1 - 128 + 1, SAMP_HALF], fp16, name=f"hq1b{h}")
                nc.gpsimd.dma_start(out=hq1bh[Q1 - 128:Q1 - 128 + 1, :],
                                    in_=onesr[:, 0:SAMP_HALF])
                hq1bs.append(hq1bh)
                hq2h = cst.tile([Q2 + 1, SAMP_HALF], fp16, name=f"hq2{h}")
                nc.gpsimd.dma_start(out=hq2h[Q2:Q2 + 1, :], in_=onesr[:, 0:SAMP_HALF])
                hq2s.append(hq2h)
                e_sbs.append(cst.tile([3, SAMP_HALF], f32, name=f"e_sb{h}"))

            # warm-up matmuls (output consumed by a cheap DVE op)
            pw = ps.tile([128, 512], f32, name="pw", tag="hp")
            for i in range(8):
                nc.tensor.matmul(pw[:, :], wtiny[:, 0:128], wtiny[:, :],
                                 start=True, stop=True)
            wsink = cst.tile([32, 2], fp16)
            nc.vector.tensor_scalar_max(wsink[:, :], pw[0:32, 0:2], 0.0)

            def tail_half(h):
                sl = slice(h * SAMP_HALF, (h + 1) * SAMP_HALF)
                xqh, hr1h, hr2h = xqs[h], hr1s[h], hr2s[h]
                hq1ah, hq1bh, hq2h, e_sb = hq1as[h], hq1bs[h], hq2s[h], e_sbs[h]

                pr1 = pst.tile([R1, SAMP_HALF], f32, name=f"pr1_{h}", tag="tail")
                nc.tensor.matmul(pr1[:, :], r1s, pooled[:, sl], start=True, stop=True)
                nc.scalar.activation(hr1h[0:R1, :], pr1[:, :], Relu)

                pr2 = pst.tile([R2, SAMP_HALF], f32, name=f"pr2_{h}", tag="tail")
                nc.tensor.matmul(pr2[:, :], r2s, hr1h[:, :], start=True, stop=True)
                nc.scalar.activation(hr2h[0:R2, :], pr2[:, :], Relu)

                pr3 = pst.tile([R3, SAMP_HALF], f32, name=f"pr3_{h}", tag="tail")
                nc.tensor.matmul(pr3[:, :], r3s, hr2h[:, :], start=True, stop=True)
                nc.scalar.copy(xqh[0:R3, :], pr3[:, :])

                pq1a = pst.tile([128, SAMP_HALF], f32, name=f"pq1a_{h}", tag="tail")
                pq1b = pst.tile([Q1 - 128, SAMP_HALF], f32, name=f"pq1b_{h}", tag="tail")
                nc.tensor.matmul(pq1a[:, :], q1as, xqh[:, :], start=True, stop=True)
                nc.tensor.matmul(pq1b[:, :], q1bs, xqh[:, :], start=True, stop=True)
                nc.scalar.activation(hq1ah[:, :], pq1a[:, :], Relu)
                nc.vector.tensor_scalar_max(hq1bh[0:Q1 - 128, :], pq1b[:, :], 0.0)

                pq2 = pst.tile([Q2, SAMP_HALF], f32, name=f"pq2_{h}", tag="tail")
                nc.tensor.matmul(pq2[:, :], q2as, hq1ah[:, :], start=True, stop=False)
                nc.tensor.matmul(pq2[:, :], q2bs, hq1bh[:, :], start=False, stop=True)
                nc.scalar.activation(hq2h[0:Q2, :], pq2[:, :], Relu)

                pq3 = pst.tile([Q3, SAMP_HALF], f32, name=f"pq3_{h}", tag="tail")
                nc.tensor.matmul(pq3[:, :], q3as, hq2h[:, :], start=True, stop=True)

                # softmax (fp32)
                nc.scalar.activation(e_sb[:, :], pq3[:, :], Exp)
                ssum = pst.tile([1, SAMP_HALF], f32, name=f"ssum{h}", tag="tail")
                nc.tensor.matmul(ssum[:, :], ones3[:, :], e_sb[:, :], start=True, stop=True)
                rec = cst.tile([1, SAMP_HALF], f32, name=f"rec{h}")
                nc.vector.reciprocal(rec[:, :], ssum[:, :])

                eT = pst.tile([128, 3], f32, name=f"eT{h}", tag="tail")
                nc.tensor.transpose(eT[:, :], e_sb[:, :], eye3s[:, :])
                rT = pst.tile([128, 1], f32, name=f"rT{h}", tag="tail")
                nc.tensor.transpose(rT[:, :], rec[:, :], eye1[:, :])
                rTs = cst.tile([128, 1], f32, name=f"rTs{h}")
                nc.vector.tensor_copy(rTs[:, :], rT[:, :])
                o_sb = cst.tile([128, 3], f32, name=f"o_sb{h}")
                nc.vector.tensor_scalar_mul(o_sb[:, :], eT[:, :], rTs[:, :])
                nc.sync.dma_start(out=out[h * SAMP_HALF:(h + 1) * SAMP_HALF, :],
                                  in_=o_sb[:, :])

            def phi_l1(h, x_sb):
                # L1 sweep: CPM chunks per tile, one per PE row-group (concurrent)
                for t in range(NPT):
                    p1 = ps.tile([128, PT], f32, name="p1", tag="hp")
                    for cc in range(CPM):
                        ch = t * CPM + cc             # global 512-chunk index
                        j = ch % 4                     # PE row-group
                        g = ch // 4                    # x column group
                        nc.tensor.matmul(p1[0:F1, cc * 512:(cc + 1) * 512],
                                         blob_sb[32 * j:32 * j + 4, 0:F1],
                                         x_sb[32 * j:32 * j + 4, g * 512:(g + 1) * 512],
                                         start=True, stop=True, tile_position=(32 * j, 0))
                    c = t * PT
                    if t % 2 == 0:
                        nc.scalar.activation(h1[0:F1, c:c + PT], p1[0:F1, :], Relu)
                    else:
                        nc.vector.tensor_scalar_max(h1[0:F1, c:c + PT], p1[0:F1, :], 0.0)
            def phi_l2(h):
                for t in range(NPT):
                    p2 = ps.tile([128, PT], f32, name="p2", tag="hp")
                    for cc in range(CPM):
                        c0 = t * PT + cc * 512
                        nc.tensor.matmul(p2[0:F2, cc * 512:(cc + 1) * 512],
                                         w2s, h1[:, c0:c0 + 512],
                                         start=True, stop=True)
                    c = t * PT
                    if t % 2 == 0:
                        nc.scalar.activation(h2[0:F2, c:c + PT], p2[0:F2, :], Relu)
                    else:
                        nc.vector.tensor_scalar_max(h2[0:F2, c:c + PT], p2[0:F2, :], 0.0)
            def phi_l3(h):
                for t in range(NPT):
                    p3 = ps.tile([128, PT], f32, name="p3", tag="hp")
                    for cc in range(CPM):
                        c0 = t * PT + cc * 512
                        nc.tensor.matmul(p3[0:F3, cc * 512:(cc + 1) * 512],
                                         w3s, h2[:, c0:c0 + 512],
                                         start=True, stop=True)
                    p3g = p3[0:F3, :].rearrange("p (g m) -> p g m", m=M)
                    h3ra = scr.tile([F3, PT // 2], fp16, name="h3ra", tag="h3ra", bufs=3)
                    h3g = h3ra[:, :].rearrange("p (g m) -> p g m", m=GS)
                    # first half of each 128-group: plain relu copy (ACT)
                    nc.scalar.activation(h3g, p3g[:, :, 0:GS], Relu)
                    # second half: relu(psum) + h3ra -> pairwise-summed (DVE)
                    sv = s_half[:, t * (PT // 2):(t + 1) * (PT // 2)]
                    nc.vector.scalar_tensor_tensor(
                        sv.rearrange("p (g m) -> p g m", m=GS),
                        p3g[:, :, GS:M], 0.0, h3g,
                        op0=Alu.max, op1=Alu.add)
            def trees(h):
                USAMP = 32
                for u in range(SAMP_HALF // USAMP):
                    w = GS
                    srt = s_half[:, u * USAMP * GS:(u + 1) * USAMP * GS]
                    lv = 0
                    while w > 1:
                        w //= 2
                        if w > 1:
                            nxt = scr.tile([F3, USAMP * w], fp16, name=f"tl{lv}",
                                           tag=f"tl{lv}", bufs=2)[:, :]
                        else:
                            nxt = pooled[0:F3, h * SAMP_HALF + u * USAMP:
                                         h * SAMP_HALF + (u + 1) * USAMP]
                        a3 = srt.rearrange("p (g m) -> p g m", m=2 * w)
                        eng = nc.gpsimd if lv == 0 else nc.vector
                        eng.tensor_tensor(
                            out=nxt.rearrange("p (g m) -> p g m", m=w) if w > 1 else nxt,
                            in0=a3[:, :, 0:w], in1=a3[:, :, w:2 * w], op=Alu.add)
                        srt = nxt
                        lv += 1

            phi_l1(0, x_sb0)
            phi_l2(0)
            phi_l3(0)
            trees(0)
            x_sb1 = xp.tile([128, HALF // 4], fp16, name="x_sb1", tag="x", bufs=2)
            for j in range(4):
                eng = nc.sync if j < 2 else nc.scalar
                eng.dma_start(out=x_sb1[32 * j:32 * j + 4, :], in_=xin[1, j])
            phi_l1(1, x_sb1)
            tail_half(0)
            phi_l2(1)
            phi_l3(1)
            trees(1)
            tail_half(1)

    nc.compile()
    return nc


def _prep_inputs(dyn, static, phi_w1, phi_b1, phi_w2, phi_b2, phi_w3, phi_b3,
                 rho_w1, rho_b1, rho_w2, rho_b2, rho_w3, rho_b3,
                 q_w1, q_b1, q_w2, q_b2, q_w3, q_b3):
    """Build the per-core input maps (host-side layout transforms, all fp16)."""
    fp16 = np.float16

    def aug_t(w, b):
        # [out, in] weight + bias -> transposed augmented [in+1, out]
        return np.concatenate([w, b[:, None]], axis=1).T.astype(fp16)

    q1 = aug_t(q_w1, q_b1)               # [44, 200]
    q2 = aug_t(q_w2, q_b2)               # [201, 100]
    parts = dict(
        w1a=aug_t(phi_w1, phi_b1), w2a=aug_t(phi_w2, phi_b2),
        w3a=aug_t(phi_w3, phi_b3), r1a=aug_t(rho_w1, rho_b1),
        r2a=aug_t(rho_w2, rho_b2), r3a=aug_t(rho_w3, rho_b3),
        q1aw=q1[:, 0:128], q1bw=q1[:, 128:],
        q2aw=q2[0:128, :], q2bw=q2[128:, :], q3aw=aug_t(q_w3, q_b3))

    eye3 = np.eye(3, dtype=np.float32)
    onesr = np.ones((1, HALF), dtype=fp16)

    base_blob = np.zeros((128, BLOBW), dtype=fp16)
    for name, (r, cc, o) in _BLOB.items():
        if name != "statt":
            base_blob[0:r, o:o + cc] = parts[name]
    for j in range(1, 4):   # replicate L1 weights into each PE row-group
        base_blob[32 * j:32 * j + 4, 0:F1] = parts["w1a"]

    in_maps = []
    for c in range(N_CORES):
        blob = base_blob.copy()
        r, cc, o = _BLOB["statt"]
        blob[0:r, o:o + cc] = static[c * BC:(c + 1) * BC].T.astype(fp16)
        xc = dyn[c * BC:(c + 1) * BC].reshape(EC, D).astype(fp16)
        # [2 halves, 4 row-groups, 4 rows(x0,x1,x2,1), HALF//4]
        xin = np.empty((2, 4, 4, HALF // 4), dtype=fp16)
        for hh in range(2):
            xh = xc[hh * HALF:(hh + 1) * HALF].reshape(HALF // 512, 512, D)
            for j in range(4):
                chunks = xh[j::4]                      # [8, 512, 3]
                blkT = chunks.transpose(2, 0, 1).reshape(D, -1)
                xin[hh, j, 0:3] = blkT
                xin[hh, j, 3] = 1.0
        in_maps.append(dict(xin=xin, blob=blob, onesr=onesr, eye3=eye3))
    return in_maps


def kernel(**inputs):
    from concourse.bass_utils import run_bass_kernel_spmd

    if "nc" not in _compiled:
        _compiled["nc"] = _build()
    nc = _compiled["nc"]

    in_maps = _prep_inputs(**inputs)
    res = run_bass_kernel_spmd(nc, in_maps, core_ids=list(range(N_CORES)))
    out = np.concatenate([res.results[c]["out"] for c in range(N_CORES)], axis=0)
    return out.astype(np.float32)


# revision 15
# speedup vs baseline: 1.0829x; 1.0583x over previous
"""Trainium2 Bass kernel for the DeepSets-style segment_reduce network.

Network (per sample, B=2048, M=128 elements):
  phi: 3 -> 120 -> 100 -> 80 MLP (all ReLU), applied per element
  pooled = sum over the 128 elements                      [B, 80]
  rho:  80 -> 60 -> 60 -> 40 (ReLU, ReLU, linear)
  q:    concat(rho_out, static) 43 -> 200 -> 100 -> 3, softmax

Mapping: data-parallel over 8 NeuronCores (256 samples each). Activations are
kept feature-major [features, elements] in SBUF so every layer is a single
stationary-weight matmul; biases are folded in as an extra contraction row
driven by a constant-ones row. All matmul operands are fp16 (fp32 PSUM
accumulation). L1 (K=4) runs 4-way row-group packed on the PE array. The
element-pool runs as a fused relu+pairwise-add (scalar_tensor_tensor) off
PSUM, then fp16 halving trees on GpSimd. The rho/q/softmax tail runs per
128-sample half so the first half's tail hides under the second half's phi.
"""

import sys
import numpy as np

sys.path.insert(0, '/opt/trn_rl_repo')

B, M, D = 2048, 128, 3
N_CORES = 8
BC = B // N_CORES            # samples per core (256)
EC = BC * M                  # elements per core (32768)
HALF = EC // 2               # elements per half-sweep (16384)
SAMP_HALF = BC // 2          # samples per half (128)
PT = 1024                    # elements per PSUM tile
NPT = HALF // PT             # psum tiles per half (16)
CPM = PT // 512              # 512-chunks per psum tile (2)
SPL = PT // 2                # ACT/DVE column split inside a [*, PT] copy
GS = M // 2                  # pair-summed group width (64)

F1, F2, F3 = 120, 100, 80    # phi widths
R1, R2, R3 = 60, 60, 40      # rho widths
Q1, Q2, Q3 = 200, 100, 3     # q widths
XQ = R3 + 3 + 1              # q input rows: rho_out + static + ones (44)

# packed weight blob column layout: name -> (rows, cols, col_offset)
_BLOB = {}
_off = 0
for _name, _r, _c in [("w1a", 4, F1), ("w2a", F1 + 1, F2), ("w3a", F2 + 1, F3),
                      ("r1a", F3 + 1, R1), ("r2a", R1 + 1, R2), ("r3a", R2 + 1, R3),
                      ("q1aw", XQ, 128), ("q1bw", XQ, Q1 - 128),
                      ("q2aw", 128, Q2), ("q2bw", Q1 - 128 + 1, Q2),
                      ("q3aw", Q2 + 1, Q3), ("statt", 3, BC)]:
    _BLOB[_name] = (_r, _c, _off)
    _off += _c
BLOBW = _off

_compiled = {}


def _build():
    import concourse.bacc as bacc
    import concourse.mybir as mybir
    from concourse import tile

    f32 = mybir.dt.float32
    fp16 = mybir.dt.float16
    Relu = mybir.ActivationFunctionType.Relu
    Exp = mybir.ActivationFunctionType.Exp
    Alu = mybir.AluOpType

    nc = bacc.Bacc("TRN2", target_bir_lowering=False, debug=False,
                   enable_asserts=False, num_devices=N_CORES)

    xin = nc.dram_tensor("xin", [2, 4, 4, HALF // 4], fp16, kind="ExternalInput").ap()
    blob = nc.dram_tensor("blob", [128, BLOBW], fp16, kind="ExternalInput").ap()
    onesr = nc.dram_tensor("onesr", [1, HALF], fp16, kind="ExternalInput").ap()
    eye3 = nc.dram_tensor("eye3", [3, 3], f32, kind="ExternalInput").ap()
    out = nc.dram_tensor("out", [BC, 3], f32, kind="ExternalOutput").ap()

    with tile.TileContext(nc) as tc:
        with tc.tile_pool(name="cst", bufs=1) as cst, \
             tc.tile_pool(name="xp", bufs=2) as xp, \
             tc.tile_pool(name="scr", bufs=1) as scr, \
             tc.tile_pool(name="ps", bufs=3, space="PSUM") as ps, \
             tc.tile_pool(name="pst", bufs=2, space="PSUM") as pst:

            # x half 0 first so the PE can start ASAP (4 row-group DMAs)
            x_sb0 = xp.tile([128, HALF // 4], fp16, name="x_sb0", tag="x", bufs=2)
            for j in range(4):
                eng = nc.sync if j < 2 else nc.scalar
                eng.dma_start(out=x_sb0[32 * j:32 * j + 4, :], in_=xin[0, j])

            blob_sb = cst.tile([128, BLOBW], fp16)
            nc.sync.dma_start(out=blob_sb[:, :], in_=blob)

            def wslice(name):
                r, c, o = _BLOB[name]
                return blob_sb[0:r, o:o + c]

            w2s, w3s = wslice("w2a"), wslice("w3a")
            r1s, r2s, r3s = wslice("r1a"), wslice("r2a"), wslice("r3a")
            q1as, q1bs = wslice("q1aw"), wslice("q1bw")
            q2as, q2bs = wslice("q2aw"), wslice("q2bw")
            q3as, statt = wslice("q3aw"), wslice("statt")

            # PE warm-up source (HAM ramp while input DMAs land)
            wtiny = cst.tile([128, 512], fp16)
            nc.vector.memset(wtiny[:, :], 0.0)
            tpre = cst.tile([1, 2], fp16)
            nc.scalar.activation(tpre[:, :], wtiny[0:1, 0:2],
                                 mybir.ActivationFunctionType.Relu)

            # persistent activation planes (ones rows written once)
            h1 = cst.tile([F1 + 1, HALF], fp16)
            h2 = cst.tile([F2 + 1, HALF], fp16)
            s_half = cst.tile([F3, HALF // 2], fp16)
            nc.gpsimd.dma_start(out=h1[F1:F1 + 1, :], in_=onesr)
            nc.gpsimd.dma_start(out=h2[F2:F2 + 1, :], in_=onesr)

            pooled = cst.tile([F3 + 1, BC], fp16)      # ones row at 80
            nc.gpsimd.dma_start(out=pooled[F3:F3 + 1, :], in_=onesr[:, 0:BC])
            eye3s = cst.tile([3, 3], f32)
            nc.gpsimd.dma_start(out=eye3s[:, :], in_=eye3)
            eye1 = cst.tile([1, 1], f32)
            nc.vector.memset(eye1[:, :], 1.0)
            ones3 = cst.tile([3, 1], f32)
            nc.vector.memset(ones3[:, :], 1.0)

            # per-half tail tiles (ones rows preloaded)
            xqs, hr1s, hr2s, hq1as, hq1bs, hq2s, e_sbs = [], [], [], [], [], [], []
            for h in range(2):
                xqh = cst.tile([XQ, SAMP_HALF], fp16, name=f"xq{h}")
                nc.gpsimd.dma_start(out=xqh[R3:R3 + 3, :],
                                    in_=statt[:, h * SAMP_HALF:(h + 1) * SAMP_HALF])
                nc.gpsimd.dma_start(out=xqh[XQ - 1:XQ, :], in_=onesr[:, 0:SAMP_HALF])
                xqs.append(xqh)
                hr1h = cst.tile([R1 + 1, SAMP_HALF], fp16, name=f"hr1{h}")
                nc.gpsimd.dma_start(out=hr1h[R1:R1 + 1, :], in_=onesr[:, 0:SAMP_HALF])
                hr1s.append(hr1h)
                hr2h = cst.tile([R2 + 1, SAMP_HALF], fp16, name=f"hr2{h}")
                nc.gpsimd.dma_start(out=hr2h[R2:R2 + 1, :], in_=onesr[:, 0:SAMP_HALF])
                hr2s.append(hr2h)
                hq1as.append(cst.tile([128, SAMP_HALF], fp16, name=f"hq1a{h}"))
                hq1bh = cst.tile([Q1 - 128 + 1, SAMP_HALF], fp16, name=f"hq1b{h}")
                nc.gpsimd.dma_start(out=hq1bh[Q1 - 128:Q1 - 128 + 1, :],
                                    in_=onesr[:, 0:SAMP_HALF])
                hq1bs.append(hq1bh)
                hq2h = cst.tile([Q2 + 1, SAMP_HALF], fp16, name=f"hq2{h}")
                nc.gpsimd.dma_start(out=hq2h[Q2:Q2 + 1, :], in_=onesr[:, 0:SAMP_HALF])
                hq2s.append(hq2h)
                e_sbs.append(cst.tile([3, SAMP_HALF], f32, name=f"e_sb{h}"))

            # warm-up matmuls (output consumed by a cheap DVE op)
            pw = ps.tile([128, 512], f32, name="pw", tag="hp")
            for i in range(8):
                nc.tensor.matmul(pw[:, :], wtiny[:, 0:128], wtiny[:, :],
                                 start=True, stop=True)
            wsink = cst.tile([32, 2], fp16)
            nc.vector.tensor_scalar_max(wsink[:, :], pw[0:32, 0:2], 0.0)

            def tail_half(h):
                sl = slice(h * SAMP_HALF, (h + 1) * SAMP_HALF)
                xqh, hr1h, hr2h = xqs[h], hr1s[h], hr2s[h]
                hq1ah, hq1bh, hq2h, e_sb = hq1as[h], hq1bs[h], hq2s[h], e_sbs[h]

                pr1 = pst.tile([R1, SAMP_HALF], f32, name=f"pr1_{h}", tag="tail")
                nc.tensor.matmul(pr1[:, :], r1s, pooled[:, sl], start=True, stop=True)
                nc.scalar.activation(hr1h[0:R1, :], pr1[:, :], Relu)

                pr2 = pst.tile([R2, SAMP_HALF], f32, name=f"pr2_{h}", tag="tail")
                nc.tensor.matmul(pr2[:, :], r2s, hr1h[:, :], start=True, stop=True)
                nc.scalar.activation(hr2h[0:R2, :], pr2[:, :], Relu)

                pr3 = pst.tile([R3, SAMP_HALF], f32, name=f"pr3_{h}", tag="tail")
                nc.tensor.matmul(pr3[:, :], r3s, hr2h[:, :], start=True, stop=True)
                nc.scalar.copy(xqh[0:R3, :], pr3[:, :])

                pq1a = pst.tile([128, SAMP_HALF], f32, name=f"pq1a_{h}", tag="tail")
                pq1b = pst.tile([Q1 - 128, SAMP_HALF], f32, name=f"pq1b_{h}", tag="tail")
                nc.tensor.matmul(pq1a[:, :], q1as, xqh[:, :], start=True, stop=True)
                nc.tensor.matmul(pq1b[:, :], q1bs, xqh[:, :], start=True, stop=True)
                nc.scalar.activation(hq1ah[:, :], pq1a[:, :], Relu)
                nc.vector.tensor_scalar_max(hq1bh[0:Q1 - 128, :], pq1b[:, :], 0.0)

                pq2 = pst.tile([Q2, SAMP_HALF], f32, name=f"pq2_{h}", tag="tail")
                nc.tensor.matmul(pq2[:, :], q2as, hq1ah[:, :], start=True, stop=False)
                nc.tensor.matmul(pq2[:, :], q2bs, hq1bh[:, :], start=False, stop=True)
                nc.scalar.activation(hq2h[0:Q2, :], pq2[:, :], Relu)

                pq3 = pst.tile([Q3, SAMP_HALF], f32, name=f"pq3_{h}", tag="tail")
                nc.tensor.matmul(pq3[:, :], q3as, hq2h[:, :], start=True, stop=True)

                # softmax (fp32)
                nc.scalar.activation(e_sb[:, :], pq3[:, :], Exp)
                ssum = pst.tile([1, SAMP_HALF], f32, name=f"ssum{h}", tag="tail")
                nc.tensor.matmul(ssum[:, :], ones3[:, :], e_sb[:, :], start=True, stop=True)
                rec = cst.tile([1, SAMP_HALF], f32, name=f"rec{h}")
                nc.vector.reciprocal(rec[:, :], ssum[:, :])

                eT = pst.tile([128, 3], f32, name=f"eT{h}", tag="tail")
                nc.tensor.transpose(eT[:, :], e_sb[:, :], eye3s[:, :])
                rT = pst.tile([128, 1], f32, name=f"rT{h}", tag="tail")
                nc.tensor.transpose(rT[:, :], rec[:, :], eye1[:, :])
                rTs = cst.tile([128, 1], f32, name=f"rTs{h}")
                nc.vector.tensor_copy(rTs[:, :], rT[:, :])
                o_sb = cst.tile([128, 3], f32, name=f"o_sb{h}")
                nc.vector.tensor_scalar_mul(o_sb[:, :], eT[:, :], rTs[:, :])
                nc.sync.dma_start(out=out[h * SAMP_HALF:(h + 1) * SAMP_HALF, :],
                                  in_=o_sb[:, :])

            def phi_l1(h, x_sb):
                # L1 sweep: CPM chunks per tile, one per PE row-group (concurrent)
                for t in range(NPT):
                    p1 = ps.tile([128, PT], f32, name="p1", tag="hp")
                    for cc in range(CPM):
                        ch = t * CPM + cc             # global 512-chunk index
                        j = ch % 4                     # PE row-group
                        g = ch // 4                    # x column group
                        nc.tensor.matmul(p1[0:F1, cc * 512:(cc + 1) * 512],
                                         blob_sb[32 * j:32 * j + 4, 0:F1],
                                         x_sb[32 * j:32 * j + 4, g * 512:(g + 1) * 512],
                                         start=True, stop=True, tile_position=(32 * j, 0))
                    c = t * PT
                    if t % 2 == 0:
                        nc.scalar.activation(h1[0:F1, c:c + PT], p1[0:F1, :], Relu)
                    else:
                        nc.vector.tensor_scalar_max(h1[0:F1, c:c + PT], p1[0:F1, :], 0.0)
            def phi_l2(h):
                for t in range(NPT):
                    p2 = ps.tile([128, PT], f32, name="p2", tag="hp")
                    for cc in range(CPM):
                        c0 = t * PT + cc * 512
                        nc.tensor.matmul(p2[0:F2, cc * 512:(cc + 1) * 512],
                                         w2s, h1[:, c0:c0 + 512],
                                         start=True, stop=True)
                    c = t * PT
                    if t % 2 == 0:
                        nc.scalar.activation(h2[0:F2, c:c + PT], p2[0:F2, :], Relu)
                    else:
                        nc.vector.tensor_scalar_max(h2[0:F2, c:c + PT], p2[0:F2, :], 0.0)
            def phi_l3(h):
                for t in range(NPT):
                    p3 = ps.tile([128, PT], f32, name="p3", tag="hp")
                    for cc in range(CPM):
                        c0 = t * PT + cc * 512
                        nc.tensor.matmul(p3[0:F3, cc * 512:(cc + 1) * 512],
                                         w3s, h2[:, c0:c0 + 512],
                                         start=True, stop=True)
                    p3g = p3[0:F3, :].rearrange("p (g m) -> p g m", m=M)
                    h3ra = scr.tile([F3, PT // 2], fp16, name="h3ra", tag="h3ra", bufs=3)
                    h3g = h3ra[:, :].rearrange("p (g m) -> p g m", m=GS)
                    # first half of each 128-group: plain relu copy (ACT)
                    nc.scalar.activation(h3g, p3g[:, :, 0:GS], Relu)
                    # second half: relu(psum) + h3ra -> pairwise-summed (DVE)
                    sv = s_half[:, t * (PT // 2):(t + 1) * (PT // 2)]
                    nc.vector.scalar_tensor_tensor(
                        sv.rearrange("p (g m) -> p g m", m=GS),
                        p3g[:, :, GS:M], 0.0, h3g,
                        op0=Alu.max, op1=Alu.add)
            t1_half = cst.tile([F3, SAMP_HALF * GS // 2], fp16)

            def trees(h):
                # level 1 per 32-sample unit on GpSimd (starts as STT tiles land)
                USAMP = 32
                W1 = GS // 2
                for u in range(SAMP_HALF // USAMP):
                    a3 = s_half[:, u * USAMP * GS:(u + 1) * USAMP * GS] \
                        .rearrange("p (g m) -> p g m", m=GS)
                    nc.gpsimd.tensor_tensor(
                        out=t1_half[:, u * USAMP * W1:(u + 1) * USAMP * W1]
                        .rearrange("p (g m) -> p g m", m=W1),
                        in0=a3[:, :, 0:W1], in1=a3[:, :, W1:GS], op=Alu.add)
                # levels 2+ once per half on DVE
                w = W1
                srt = t1_half[:, :]
                lv = 0
                while w > 1:
                    w //= 2
                    if w > 1:
                        nxt = scr.tile([F3, SAMP_HALF * w], fp16, name=f"tl{lv}",
                                       tag=f"tl{lv}", bufs=2)[:, :]
                    else:
                        nxt = pooled[0:F3, h * SAMP_HALF:(h + 1) * SAMP_HALF]
                    a3 = srt.rearrange("p (g m) -> p g m", m=2 * w)
                    nc.vector.tensor_tensor(
                        out=nxt.rearrange("p (g m) -> p g m", m=w) if w > 1 else nxt,
                        in0=a3[:, :, 0:w], in1=a3[:, :, w:2 * w], op=Alu.add)
                    srt = nxt
                    lv += 1

            phi_l1(0, x_sb0)
            phi_l2(0)
            phi_l3(0)
            trees(0)
            x_sb1 = xp.tile([128, HALF // 4], fp16, name="x_sb1", tag="x", bufs=2)
            for j in range(4):
                eng = nc.sync if j < 2 else nc.scalar
                eng.dma_start(out=x_sb1[32 * j:32 * j + 4, :], in_=xin[1, j])
            phi_l1(1, x_sb1)
            tail_half(0)
            phi_l2(1)
            phi_l3(1)
            trees(1)
            tail_half(1)

    nc.compile()
    return nc


def _prep_inputs(dyn, static, phi_w1, phi_b1, phi_w2, phi_b2, phi_w3, phi_b3,
                 rho_w1, rho_b1, rho_w2, rho_b2, rho_w3, rho_b3,
                 q_w1, q_b1, q_w2, q_b2, q_w3, q_b3):
    """Build the per-core input maps (host-side layout transforms, all fp16)."""
    fp16 = np.float16

    def aug_t(w, b):
        # [out, in] weight + bias -> transposed augmented [in+1, out]
        return np.concatenate([w, b[:, None]], axis=1).T.astype(fp16)

    q1 = aug_t(q_w1, q_b1)               # [44, 200]
    q2 = aug_t(q_w2, q_b2)               # [201, 100]
    parts = dict(
        w1a=aug_t(phi_w1, phi_b1), w2a=aug_t(phi_w2, phi_b2),
        w3a=aug_t(phi_w3, phi_b3), r1a=aug_t(rho_w1, rho_b1),
        r2a=aug_t(rho_w2, rho_b2), r3a=aug_t(rho_w3, rho_b3),
        q1aw=q1[:, 0:128], q1bw=q1[:, 128:],
        q2aw=q2[0:128, :], q2bw=q2[128:, :], q3aw=aug_t(q_w3, q_b3))

    eye3 = np.eye(3, dtype=np.float32)
    onesr = np.ones((1, HALF), dtype=fp16)

    base_blob = np.zeros((128, BLOBW), dtype=fp16)
    for name, (r, cc, o) in _BLOB.items():
        if name != "statt":
            base_blob[0:r, o:o + cc] = parts[name]
    for j in range(1, 4):   # replicate L1 weights into each PE row-group
        base_blob[32 * j:32 * j + 4, 0:F1] = parts["w1a"]

    in_maps = []
    for c in range(N_CORES):
        blob = base_blob.copy()
        r, cc, o = _BLOB["statt"]
        blob[0:r, o:o + cc] = static[c * BC:(c + 1) * BC].T.astype(fp16)
        xc = dyn[c * BC:(c + 1) * BC].reshape(EC, D).astype(fp16)
        # [2 halves, 4 row-groups, 4 rows(x0,x1,x2,1), HALF//4]
        xin = np.empty((2, 4, 4, HALF // 4), dtype=fp16)
        for hh in range(2):
            xh = xc[hh * HALF:(hh + 1) * HALF].reshape(HALF // 512, 512, D)
            for j in range(4):
                chunks = xh[j::4]                      # [8, 512, 3]
                blkT = chunks.transpose(2, 0, 1).reshape(D, -1)
                xin[hh, j, 0:3] = blkT
                xin[hh, j, 3] = 1.0
        in_maps.append(dict(xin=xin, blob=blob, onesr=onesr, eye3=eye3))
    return in_maps


def kernel(**inputs):
    from concourse.bass_utils import run_bass_kernel_spmd

    if "nc" not in _compiled:
        _compiled["nc"] = _build()
    nc = _compiled["nc"]

    in_maps = _prep_inputs(**inputs)
    res = run_bass_kernel_spmd(nc, in_maps, core_ids=list(range(N_CORES)))
    out = np.concatenate([res.results[c]["out"] for c in range(N_CORES)], axis=0)
    return out.astype(np.float32)


# revision 16
# speedup vs baseline: 1.1107x; 1.0257x over previous
"""Trainium2 Bass kernel for the DeepSets-style segment_reduce network.

Network (per sample, B=2048, M=128 elements):
  phi: 3 -> 120 -> 100 -> 80 MLP (all ReLU), applied per element
  pooled = sum over the 128 elements                      [B, 80]
  rho:  80 -> 60 -> 60 -> 40 (ReLU, ReLU, linear)
  q:    concat(rho_out, static) 43 -> 200 -> 100 -> 3, softmax

Mapping: data-parallel over 8 NeuronCores (256 samples each). Activations are
kept feature-major [features, elements] in SBUF so every layer is a single
stationary-weight matmul; biases are folded in as an extra contraction row
driven by a constant-ones row. All matmul operands are fp16 (fp32 PSUM
accumulation). L1 (K=4) runs 4-way row-group packed on the PE array. The
element-pool runs as a fused relu+pairwise-add (scalar_tensor_tensor) off
PSUM, then fp16 halving trees on GpSimd. The rho/q/softmax tail runs per
128-sample half so the first half's tail hides under the second half's phi.
"""

import sys
import numpy as np

sys.path.insert(0, '/opt/trn_rl_repo')

B, M, D = 2048, 128, 3
N_CORES = 8
BC = B // N_CORES            # samples per core (256)
EC = BC * M                  # elements per core (32768)
HALF = EC // 2               # elements per half-sweep (16384)
SAMP_HALF = BC // 2          # samples per half (128)
PT = 1024                    # elements per PSUM tile
NPT = HALF // PT             # psum tiles per half (16)
CPM = PT // 512              # 512-chunks per psum tile (2)
SPL = PT // 2                # ACT/DVE column split inside a [*, PT] copy
GS = M // 2                  # pair-summed group width (64)

F1, F2, F3 = 120, 100, 80    # phi widths
R1, R2, R3 = 60, 60, 40      # rho widths
Q1, Q2, Q3 = 200, 100, 3     # q widths
XQ = R3 + 3 + 1              # q input rows: rho_out + static + ones (44)

# packed weight blob column layout: name -> (rows, cols, col_offset)
_BLOB = {}
_off = 0
for _name, _r, _c in [("w1a", 4, F1), ("w2a", F1 + 1, F2), ("w3a", F2 + 1, F3),
                      ("r1a", F3 + 1, R1), ("r2a", R1 + 1, R2), ("r3a", R2 + 1, R3),
                      ("q1aw", XQ, 128), ("q1bw", XQ, Q1 - 128),
                      ("q2aw", 128, Q2), ("q2bw", Q1 - 128 + 1, Q2),
                      ("q3aw", Q2 + 1, Q3), ("statt", 3, BC)]:
    _BLOB[_name] = (_r, _c, _off)
    _off += _c
BLOBW = _off

_compiled = {}


def _build():
    import concourse.bacc as bacc
    import concourse.mybir as mybir
    from concourse import tile

    f32 = mybir.dt.float32
    fp16 = mybir.dt.float16
    Relu = mybir.ActivationFunctionType.Relu
    Exp = mybir.ActivationFunctionType.Exp
    Alu = mybir.AluOpType

    nc = bacc.Bacc("TRN2", target_bir_lowering=False, debug=False,
                   enable_asserts=False, num_devices=N_CORES)

    xin = nc.dram_tensor("xin", [2, 4, 4, HALF // 4], fp16, kind="ExternalInput").ap()
    blob = nc.dram_tensor("blob", [128, BLOBW], fp16, kind="ExternalInput").ap()
    onesr = nc.dram_tensor("onesr", [1, HALF], fp16, kind="ExternalInput").ap()
    eye3 = nc.dram_tensor("eye3", [3, 3], f32, kind="ExternalInput").ap()
    out = nc.dram_tensor("out", [BC, 3], f32, kind="ExternalOutput").ap()

    with tile.TileContext(nc) as tc:
        with tc.tile_pool(name="cst", bufs=1) as cst, \
             tc.tile_pool(name="xp", bufs=2) as xp, \
             tc.tile_pool(name="scr", bufs=1) as scr, \
             tc.tile_pool(name="ps", bufs=3, space="PSUM") as ps, \
             tc.tile_pool(name="pst", bufs=2, space="PSUM") as pst:

            # x half 0 first so the PE can start ASAP (4 row-group DMAs)
            x_sb0 = xp.tile([128, HALF // 4], fp16, name="x_sb0", tag="x", bufs=2)
            for j in range(4):
                eng = nc.sync if j < 2 else nc.scalar
                eng.dma_start(out=x_sb0[32 * j:32 * j + 4, :], in_=xin[0, j])

            blob_sb = cst.tile([128, BLOBW], fp16)
            nc.sync.dma_start(out=blob_sb[:, :], in_=blob)

            def wslice(name):
                r, c, o = _BLOB[name]
                return blob_sb[0:r, o:o + c]

            w2s, w3s = wslice("w2a"), wslice("w3a")
            r1s, r2s, r3s = wslice("r1a"), wslice("r2a"), wslice("r3a")
            q1as, q1bs = wslice("q1aw"), wslice("q1bw")
            q2as, q2bs = wslice("q2aw"), wslice("q2bw")
            q3as, statt = wslice("q3aw"), wslice("statt")

            # PE warm-up source (HAM ramp while input DMAs land)
            wtiny = cst.tile([128, 512], fp16)
            nc.vector.memset(wtiny[:, :], 0.0)
            tpre = cst.tile([1, 2], fp16)
            nc.scalar.activation(tpre[:, :], wtiny[0:1, 0:2],
                                 mybir.ActivationFunctionType.Relu)

            # persistent activation planes (ones rows written once)
            h1 = cst.tile([F1 + 1, HALF], fp16)
            h2 = cst.tile([F2 + 1, HALF], fp16)
            s_half = cst.tile([F3, HALF // 2], fp16)
            nc.gpsimd.dma_start(out=h1[F1:F1 + 1, :], in_=onesr)
            nc.gpsimd.dma_start(out=h2[F2:F2 + 1, :], in_=onesr)

            pooled = cst.tile([F3 + 1, BC], fp16)      # ones row at 80
            nc.gpsimd.dma_start(out=pooled[F3:F3 + 1, :], in_=onesr[:, 0:BC])
            eye3s = cst.tile([3, 3], f32)
            nc.gpsimd.dma_start(out=eye3s[:, :], in_=eye3)
            eye1 = cst.tile([1, 1], f32)
            nc.vector.memset(eye1[:, :], 1.0)
            ones3 = cst.tile([3, 1], f32)
            nc.vector.memset(ones3[:, :], 1.0)

            # per-half tail tiles (ones rows preloaded)
            xqs, hr1s, hr2s, hq1as, hq1bs, hq2s, e_sbs = [], [], [], [], [], [], []
            for h in range(2):
                xqh = cst.tile([XQ, SAMP_HALF], fp16, name=f"xq{h}")
                nc.gpsimd.dma_start(out=xqh[R3:R3 + 3, :],
                                    in_=statt[:, h * SAMP_HALF:(h + 1) * SAMP_HALF])
                nc.gpsimd.dma_start(out=xqh[XQ - 1:XQ, :], in_=onesr[:, 0:SAMP_HALF])
                xqs.append(xqh)
                hr1h = cst.tile([R1 + 1, SAMP_HALF], fp16, name=f"hr1{h}")
                nc.gpsimd.dma_start(out=hr1h[R1:R1 + 1, :], in_=onesr[:, 0:SAMP_HALF])
                hr1s.append(hr1h)
                hr2h = cst.tile([R2 + 1, SAMP_HALF], fp16, name=f"hr2{h}")
                nc.gpsimd.dma_start(out=hr2h[R2:R2 + 1, :], in_=onesr[:, 0:SAMP_HALF])
                hr2s.append(hr2h)
                hq1as.append(cst.tile([128, SAMP_HALF], fp16, name=f"hq1a{h}"))
                hq1bh = cst.tile([Q1 - 128 + 1, SAMP_HALF], fp16, name=f"hq1b{h}")
                nc.gpsimd.dma_start(out=hq1bh[Q1 - 128:Q1 - 128 + 1, :],
                                    in_=onesr[:, 0:SAMP_HALF])
                hq1bs.append(hq1bh)
                hq2h = cst.tile([Q2 + 1, SAMP_HALF], fp16, name=f"hq2{h}")
                nc.gpsimd.dma_start(out=hq2h[Q2:Q2 + 1, :], in_=onesr[:, 0:SAMP_HALF])
                hq2s.append(hq2h)
                e_sbs.append(cst.tile([3, SAMP_HALF], f32, name=f"e_sb{h}"))

            # warm-up matmuls (output consumed by a cheap DVE op)
            pw = ps.tile([128, 512], f32, name="pw", tag="hp")
            for i in range(6):
                nc.tensor.matmul(pw[:, :], wtiny[:, 0:128], wtiny[:, :],
                                 start=True, stop=True)
            wsink = cst.tile([32, 2], fp16)
            nc.vector.tensor_scalar_max(wsink[:, :], pw[0:32, 0:2], 0.0)

            def tail_half(h):
                sl = slice(h * SAMP_HALF, (h + 1) * SAMP_HALF)
                xqh, hr1h, hr2h = xqs[h], hr1s[h], hr2s[h]
                hq1ah, hq1bh, hq2h, e_sb = hq1as[h], hq1bs[h], hq2s[h], e_sbs[h]

                pr1 = pst.tile([R1, SAMP_HALF], f32, name=f"pr1_{h}", tag="tail")
                nc.tensor.matmul(pr1[:, :], r1s, pooled[:, sl], start=True, stop=True)
                nc.scalar.activation(hr1h[0:R1, :], pr1[:, :], Relu)

                pr2 = pst.tile([R2, SAMP_HALF], f32, name=f"pr2_{h}", tag="tail")
                nc.tensor.matmul(pr2[:, :], r2s, hr1h[:, :], start=True, stop=True)
                nc.scalar.activation(hr2h[0:R2, :], pr2[:, :], Relu)

                pr3 = pst.tile([R3, SAMP_HALF], f32, name=f"pr3_{h}", tag="tail")
                nc.tensor.matmul(pr3[:, :], r3s, hr2h[:, :], start=True, stop=True)
                nc.scalar.copy(xqh[0:R3, :], pr3[:, :])

                pq1a = pst.tile([128, SAMP_HALF], f32, name=f"pq1a_{h}", tag="tail")
                pq1b = pst.tile([Q1 - 128, SAMP_HALF], f32, name=f"pq1b_{h}", tag="tail")
                nc.tensor.matmul(pq1a[:, :], q1as, xqh[:, :], start=True, stop=True)
                nc.tensor.matmul(pq1b[:, :], q1bs, xqh[:, :], start=True, stop=True)
                nc.scalar.activation(hq1ah[:, :], pq1a[:, :], Relu)
                nc.vector.tensor_scalar_max(hq1bh[0:Q1 - 128, :], pq1b[:, :], 0.0)

                pq2 = pst.tile([Q2, SAMP_HALF], f32, name=f"pq2_{h}", tag="tail")
                nc.tensor.matmul(pq2[:, :], q2as, hq1ah[:, :], start=True, stop=False)
                nc.tensor.matmul(pq2[:, :], q2bs, hq1bh[:, :], start=False, stop=True)
                nc.scalar.activation(hq2h[0:Q2, :], pq2[:, :], Relu)

                pq3 = pst.tile([Q3, SAMP_HALF], f32, name=f"pq3_{h}", tag="tail")
                nc.tensor.matmul(pq3[:, :], q3as, hq2h[:, :], start=True, stop=True)

                # softmax (fp32)
                nc.scalar.activation(e_sb[:, :], pq3[:, :], Exp)
                ssum = pst.tile([1, SAMP_HALF], f32, name=f"ssum{h}", tag="tail")
                nc.tensor.matmul(ssum[:, :], ones3[:, :], e_sb[:, :], start=True, stop=True)
                rec = cst.tile([1, SAMP_HALF], f32, name=f"rec{h}")
                nc.vector.reciprocal(rec[:, :], ssum[:, :])

                eT = pst.tile([128, 3], f32, name=f"eT{h}", tag="tail")
                nc.tensor.transpose(eT[:, :], e_sb[:, :], eye3s[:, :])
                rT = pst.tile([128, 1], f32, name=f"rT{h}", tag="tail")
                nc.tensor.transpose(rT[:, :], rec[:, :], eye1[:, :])
                rTs = cst.tile([128, 1], f32, name=f"rTs{h}")
                nc.vector.tensor_copy(rTs[:, :], rT[:, :])
                o_sb = cst.tile([128, 3], f32, name=f"o_sb{h}")
                nc.vector.tensor_scalar_mul(o_sb[:, :], eT[:, :], rTs[:, :])
                nc.sync.dma_start(out=out[h * SAMP_HALF:(h + 1) * SAMP_HALF, :],
                                  in_=o_sb[:, :])

            def phi_l1(h, x_sb):
                # L1 sweep: CPM chunks per tile, one per PE row-group (concurrent)
                for t in range(NPT):
                    p1 = ps.tile([128, PT], f32, name="p1", tag="hp")
                    for cc in range(CPM):
                        ch = t * CPM + cc             # global 512-chunk index
                        j = ch % 4                     # PE row-group
                        g = ch // 4                    # x column group
                        nc.tensor.matmul(p1[0:F1, cc * 512:(cc + 1) * 512],
                                         blob_sb[32 * j:32 * j + 4, 0:F1],
                                         x_sb[32 * j:32 * j + 4, g * 512:(g + 1) * 512],
                                         start=True, stop=True, tile_position=(32 * j, 0))
                    c = t * PT
                    if t % 2 == 0:
                        nc.scalar.activation(h1[0:F1, c:c + PT], p1[0:F1, :], Relu)
                    else:
                        nc.vector.tensor_scalar_max(h1[0:F1, c:c + PT], p1[0:F1, :], 0.0)
            def phi_l2(h):
                for t in range(NPT):
                    p2 = ps.tile([128, PT], f32, name="p2", tag="hp")
                    for cc in range(CPM):
                        c0 = t * PT + cc * 512
                        nc.tensor.matmul(p2[0:F2, cc * 512:(cc + 1) * 512],
                                         w2s, h1[:, c0:c0 + 512],
                                         start=True, stop=True)
                    c = t * PT
                    if t % 2 == 0:
                        nc.scalar.activation(h2[0:F2, c:c + PT], p2[0:F2, :], Relu)
                    else:
                        nc.vector.tensor_scalar_max(h2[0:F2, c:c + PT], p2[0:F2, :], 0.0)
            def phi_l3(h):
                for t in range(NPT):
                    p3 = ps.tile([128, PT], f32, name="p3", tag="hp")
                    for cc in range(CPM):
                        c0 = t * PT + cc * 512
                        nc.tensor.matmul(p3[0:F3, cc * 512:(cc + 1) * 512],
                                         w3s, h2[:, c0:c0 + 512],
                                         start=True, stop=True)
                    p3g = p3[0:F3, :].rearrange("p (g m) -> p g m", m=M)
                    h3ra = scr.tile([F3, PT // 2], fp16, name="h3ra", tag="h3ra", bufs=3)
                    h3g = h3ra[:, :].rearrange("p (g m) -> p g m", m=GS)
                    # first half of each 128-group: plain relu copy (ACT)
                    nc.scalar.activation(h3g, p3g[:, :, 0:GS], Relu)
                    # second half: relu(psum) + h3ra -> pairwise-summed (DVE)
                    sv = s_half[:, t * (PT // 2):(t + 1) * (PT // 2)]
                    nc.vector.scalar_tensor_tensor(
                        sv.rearrange("p (g m) -> p g m", m=GS),
                        p3g[:, :, GS:M], 0.0, h3g,
                        op0=Alu.max, op1=Alu.add)
            t1_half = cst.tile([F3, SAMP_HALF * GS // 2], fp16)

            def trees(h):
                # level 1 per 32-sample unit on GpSimd (starts as STT tiles land)
                USAMP = 32
                W1 = GS // 2
                nu = SAMP_HALF // USAMP
                for u in range(nu):
                    a3 = s_half[:, u * USAMP * GS:(u + 1) * USAMP * GS] \
                        .rearrange("p (g m) -> p g m", m=GS)
                    eng = nc.vector if u == nu - 1 else nc.gpsimd
                    eng.tensor_tensor(
                        out=t1_half[:, u * USAMP * W1:(u + 1) * USAMP * W1]
                        .rearrange("p (g m) -> p g m", m=W1),
                        in0=a3[:, :, 0:W1], in1=a3[:, :, W1:GS], op=Alu.add)
                # levels 2+ once per half on DVE
                w = W1
                srt = t1_half[:, :]
                lv = 0
                while w > 1:
                    w //= 2
                    if w > 1:
                        nxt = scr.tile([F3, SAMP_HALF * w], fp16, name=f"tl{lv}",
                                       tag=f"tl{lv}", bufs=2)[:, :]
                    else:
                        nxt = pooled[0:F3, h * SAMP_HALF:(h + 1) * SAMP_HALF]
                    a3 = srt.rearrange("p (g m) -> p g m", m=2 * w)
                    nc.vector.tensor_tensor(
                        out=nxt.rearrange("p (g m) -> p g m", m=w) if w > 1 else nxt,
                        in0=a3[:, :, 0:w], in1=a3[:, :, w:2 * w], op=Alu.add)
                    srt = nxt
                    lv += 1

            phi_l1(0, x_sb0)
            phi_l2(0)
            phi_l3(0)
            trees(0)
            x_sb1 = xp.tile([128, HALF // 4], fp16, name="x_sb1", tag="x", bufs=2)
            for j in range(4):
                eng = nc.sync if j < 2 else nc.scalar
                eng.dma_start(out=x_sb1[32 * j:32 * j + 4, :], in_=xin[1, j])
            phi_l1(1, x_sb1)
            tail_half(0)
            phi_l2(1)
            phi_l3(1)
            trees(1)
            tail_half(1)

    nc.compile()
    return nc


def _prep_inputs(dyn, static, phi_w1, phi_b1, phi_w2, phi_b2, phi_w3, phi_b3,
                 rho_w1, rho_b1, rho_w2, rho_b2, rho_w3, rho_b3,
                 q_w1, q_b1, q_w2, q_b2, q_w3, q_b3):
    """Build the per-core input maps (host-side layout transforms, all fp16)."""
    fp16 = np.float16

    def aug_t(w, b):
        # [out, in] weight + bias -> transposed augmented [in+1, out]
        return np.concatenate([w, b[:, None]], axis=1).T.astype(fp16)

    q1 = aug_t(q_w1, q_b1)               # [44, 200]
    q2 = aug_t(q_w2, q_b2)               # [201, 100]
    parts = dict(
        w1a=aug_t(phi_w1, phi_b1), w2a=aug_t(phi_w2, phi_b2),
        w3a=aug_t(phi_w3, phi_b3), r1a=aug_t(rho_w1, rho_b1),
        r2a=aug_t(rho_w2, rho_b2), r3a=aug_t(rho_w3, rho_b3),
        q1aw=q1[:, 0:128], q1bw=q1[:, 128:],
        q2aw=q2[0:128, :], q2bw=q2[128:, :], q3aw=aug_t(q_w3, q_b3))

    eye3 = np.eye(3, dtype=np.float32)
    onesr = np.ones((1, HALF), dtype=fp16)

    base_blob = np.zeros((128, BLOBW), dtype=fp16)
    for name, (r, cc, o) in _BLOB.items():
        if name != "statt":
            base_blob[0:r, o:o + cc] = parts[name]
    for j in range(1, 4):   # replicate L1 weights into each PE row-group
        base_blob[32 * j:32 * j + 4, 0:F1] = parts["w1a"]

    in_maps = []
    for c in range(N_CORES):
        blob = base_blob.copy()
        r, cc, o = _BLOB["statt"]
        blob[0:r, o:o + cc] = static[c * BC:(c + 1) * BC].T.astype(fp16)
        xc = dyn[c * BC:(c + 1) * BC].reshape(EC, D).astype(fp16)
        # [2 halves, 4 row-groups, 4 rows(x0,x1,x2,1), HALF//4]
        xin = np.empty((2, 4, 4, HALF // 4), dtype=fp16)
        for hh in range(2):
            xh = xc[hh * HALF:(hh + 1) * HALF].reshape(HALF // 512, 512, D)
            for j in range(4):
                chunks = xh[j::4]                      # [8, 512, 3]
                blkT = chunks.transpose(2, 0, 1).reshape(D, -1)
                xin[hh, j, 0:3] = blkT
                xin[hh, j, 3] = 1.0
        in_maps.append(dict(xin=xin, blob=blob, onesr=onesr, eye3=eye3))
    return in_maps


def kernel(**inputs):
    from concourse.bass_utils import run_bass_kernel_spmd

    if "nc" not in _compiled:
        _compiled["nc"] = _build()
    nc = _compiled["nc"]

    in_maps = _prep_inputs(**inputs)
    res = run_bass_kernel_spmd(nc, in_maps, core_ids=list(range(N_CORES)))
    out = np.concatenate([res.results[c]["out"] for c in range(N_CORES)], axis=0)
    return out.astype(np.float32)
